# revision 9
# baseline (speedup 1.0000x reference)
"""MultiHeadAttention Trainium2 kernel.

Full inputs: x [4, 2048, 768] f32, W_qkv [2304, 768], W_proj [768, 768],
b_proj [768]. Output [4, 2048, 768] f32.

Sharding: 8 cores = 4 batches x 2 head-groups (6 heads each).
Per-core inputs (host-prepared, transposed on host):
  xT  [768, 2048]  = x[b].T
  wT  [768, 1152]  = concat(Wq_g, Wk_g, Wv_g).T   (g = head group rows)
  wpT [384, 768]   = W_proj[:, g-cols].T
Per-core output: outp [2048, 768] = partial projection output for batch b.
Host: out[b] = outp[2b] + outp[2b+1] + b_proj.

On-device (per core), ACT(exp)-bound pipeline:
  The softmax exp volume (6 heads x 2048^2 = 25.2M elems) makes the scalar
  engine the bottleneck (~220us at 1 elem/cycle + per-call overhead), so the
  design keeps ACT 100% busy and hides ALL matmul work underneath it:
  - QK energies: two heads computed CONCURRENTLY on the PE array via row
    tiling (K=64 each: head 2m on array rows 0-63, head 2m+1 on rows 64-127;
    tile_position auto-derived from base partitions).
  - AV: two heads concurrent via column tiling (M=64 each: outputs at psum
    partitions 0-63 / 64-127 of one bank).
  - softmax denominators: four M=1 ones-matmuls (2 heads x 2 q-blocks) run
    concurrently in one PE pass via column tiling at positions 0/32/64/96.
  - PSUM (8 banks): e_ps 2x2 (double-buffered exp staging) + av 2 + denom 1
    + 1 filler bank used to interleave next-pair QKV projections and
    prev-pair output projections into the PE gaps of the ACT-bound stream.
  Loop: for pair m (2 heads): for q-half: for kk (16 key blocks of 128):
    QK pair -> exp (2 ACT calls of [128,2,512]) -> AV pair + denom pass.
  Normalization: drain av/denoms (DVE), reciprocal, gpsimd partition
  broadcast, multiply into attT. Projection out = attT.T @ wpT per s-block.
"""

import ml_dtypes
import numpy as np

import concourse.bass as bass
import concourse.tile as tile
from concourse import bacc, mybir
from concourse.bass_utils import run_bass_kernel_spmd

EMB = 768
N = 2048
B = 4
D = 64
HL = 6            # heads per core
HD = HL * D       # 384 local head-dim columns
NCORES = 8
SCALE = D ** -0.5

F32 = mybir.dt.float32
BF16 = mybir.dt.bfloat16

EC = EMB // 128   # 6 emb chunks
MC = HD // 128    # 3 local head-dim chunks (= head pairs)
NQ = N // 512     # 4 query chunks of 512
NK = N // 128     # 16 key/seq chunks of 128

EXP = mybir.ActivationFunctionType.Exp


def _emit(tc):
    from contextlib import ExitStack

    nc = tc.nc
    xT = nc.dram_tensor("xT", [EMB, N], BF16, kind="ExternalInput").ap()
    wT = nc.dram_tensor("wT", [EMB, 3 * HD], BF16, kind="ExternalInput").ap()
    wpT = nc.dram_tensor("wpT", [HD, EMB], BF16, kind="ExternalInput").ap()
    outp = nc.dram_tensor("outp", [N, EMB], F32, kind="ExternalOutput").ap()

    xTr = xT.rearrange("(c p) s -> p c s", p=128)
    wTr = wT.rearrange("(c p) s -> p c s", p=128)
    wpTr = wpT.rearrange("(m p) e -> p m e", p=128)
    outr = outp.rearrange("(s p) e -> p s e", p=128)

    with ExitStack() as persist:
        ppool = persist.enter_context(tc.tile_pool(name="persist", bufs=1))
        # PE warmup junk matmuls run during the input-DMA wait and open the
        # HAM clock-gate before real work
        warm_sb = ppool.tile([128, 640], BF16)
        nc.vector.memset(warm_sb[:], 1.0)
        ones_sb = ppool.tile([128, 1], BF16)
        nc.vector.memset(ones_sb[:], 1.0)

        wp_sb = ppool.tile([128, MC, EMB], BF16)
        nc.sync.dma_start(wp_sb[:], wpTr)
        x_sb = ppool.tile([128, EC, N], BF16)
        w_sb = ppool.tile([128, EC, 3 * HD], BF16)
        for c in range(EC):
            nc.sync.dma_start(w_sb[:, c, :], wTr[:, c, :])
            nc.sync.dma_start(x_sb[:, c, :], xTr[:, c, :])

        # paired layouts: chunk m holds head 2m on partitions 0-63 and head
        # 2m+1 on partitions 64-127 (both q and k; v is [seq, 6*64])
        qT_sb = ppool.tile([128, MC, N], BF16)
        kT_sb = ppool.tile([128, MC, N], BF16)
        v_sb = ppool.tile([128, NK, HD], BF16)
        attT_sb = ppool.tile([128, MC, N], BF16)

        psum_pool = persist.enter_context(
            tc.tile_pool(name="psum", bufs=1, space="PSUM"))
        esb_pool = persist.enter_context(tc.tile_pool(name="esb", bufs=4))
        sm_pool = persist.enter_context(tc.tile_pool(name="sm", bufs=2))
        osb_pool = persist.enter_context(tc.tile_pool(name="osb", bufs=3))

        warm_ps = psum_pool.tile([128, 512], F32, tag="fill", bufs=1,
                                 name="warm_ps")
        for wi in range(16):
            nc.tensor.matmul(warm_ps[:], warm_sb[:, 0:128], warm_sb[:, 128:640],
                             start=(wi == 0), stop=(wi == 15))

        # ---------- phase-1 building blocks (also used as fillers) ----------
        def qk_block(which, m, n, tag="fill", bufs=1):
            # qT/kT chunk m, q-block n: psum [128, 512] accumulated over EC
            lo = which * HD + m * 128
            ns = slice(n * 512, (n + 1) * 512)
            mm = psum_pool.tile([128, 512], F32, tag=tag, bufs=bufs,
                                name=f"mm_{which}_{m}_{n}")
            for c in range(EC):
                nc.tensor.matmul(mm[:], w_sb[:, c, lo:lo + 128],
                                 x_sb[:, c, ns],
                                 start=(c == 0), stop=(c == EC - 1))
            dst = qT_sb if which == 0 else kT_sb
            nc.vector.tensor_copy(dst[:, m, ns], mm[:])

        def v_block(p, s, tag="fill", bufs=1):
            # v for pair p, seq block s: [128 seq, 128 vdims]
            vv = psum_pool.tile([128, 512], F32, tag=tag, bufs=bufs,
                                name=f"vv_{p}_{s}")[:, 0:128]
            for c in range(EC):
                nc.tensor.matmul(
                    vv[:],
                    x_sb[:, c, s * 128:(s + 1) * 128],
                    w_sb[:, c, 2 * HD + p * 128:2 * HD + (p + 1) * 128],
                    start=(c == 0), stop=(c == EC - 1))
            nc.vector.tensor_copy(v_sb[:, s, p * 128:(p + 1) * 128], vv[:])

        def proj_block(s):
            # out[s-block, :] = sum_m attT[:, m, sblk].T @ wp[m]
            o_sb = osb_pool.tile([128, EMB], F32, tag="osb", name=f"osb_{s}")
            ss = slice(s * 128, (s + 1) * 128)
            for half in range(2):
                pr = psum_pool.tile([128, 512], F32, tag="fill", bufs=1,
                                    name=f"pr_{s}_{half}")[:, 0:HD]
                for m in range(MC):
                    nc.tensor.matmul(
                        pr[:], attT_sb[:, m, ss],
                        wp_sb[:, m, half * HD:(half + 1) * HD],
                        start=(m == 0), stop=(m == MC - 1))
                nc.vector.tensor_copy(o_sb[:, half * HD:(half + 1) * HD],
                                      pr[:])
            nc.sync.dma_start(outr[:, s, :], o_sb[:])

        # ---------- startup: minimum work to begin pair-0 attention ----------
        # attention psum banks are still free; use them for 2-deep pipelining
        for n in range(NQ):
            qk_block(1, 0, n, tag="av", bufs=2)  # kT chunk 0 (all kk needed)
        qk_block(0, 0, 0, tag="av", bufs=2)      # qT chunk 0, first q-half
        qk_block(0, 0, 1, tag="av", bufs=2)
        v_block(0, 0, tag="dps", bufs=1)
        v_block(0, 1, tag="av", bufs=2)

        # filler queue: each entry emits one small PE group when pumped
        fillers = [lambda: qk_block(0, 0, 2), lambda: qk_block(0, 0, 3)]
        for s in range(2, NK):
            fillers.append(lambda s=s: v_block(0, s))

        def pair_fillers(p):
            out = []
            for n in range(NQ):
                out.append(lambda n=n: qk_block(1, p, n))
            for n in range(NQ):
                out.append(lambda n=n: qk_block(0, p, n))
            for s in range(NK):
                out.append(lambda s=s: v_block(p, s))
            return out

        def pump(k):
            for _ in range(k):
                if fillers:
                    fillers.pop(0)()

        # ---------- attention: ACT-bound stream ----------
        for m in range(MC):
            if m + 1 < MC:
                fillers.extend(pair_fillers(m + 1))
            for qh in range(2):
                first_unit = m == 0 and qh == 0
                if m == MC - 1 and qh == 1:
                    # proj for the first q-half fills the last unit's gaps
                    for s in range(NK // 2):
                        fillers.append(lambda s=s: proj_block(s))
                avs = [psum_pool.tile([128, 512], F32, tag="av", bufs=2,
                                      name=f"av_{m}_{qh}_{qb}")
                       for qb in range(2)]
                d_ps = psum_pool.tile([128, 512], F32, tag="dps", bufs=1,
                                      name=f"dps_{m}_{qh}")
                for kk in range(NK):
                    # no fillers during the first kks of a unit: the previous
                    # unit's normalize burst must drain from the DVE queue
                    # before a filler's psum-WAR can clear quickly
                    if first_unit:
                        pump(1)
                    elif 4 <= kk:
                        pump(1 if kk % 2 else 2)
                    ks = slice(kk * 128, (kk + 1) * 128)
                    first, last = kk == 0, kk == NK - 1
                    e_ps, e_sbs = [], []
                    for qb in range(2):
                        qs = slice((2 * qh + qb) * 512, (2 * qh + qb + 1) * 512)
                        eps = psum_pool.tile([128, 2, 512], F32, tag="eps",
                                             bufs=2,
                                             name=f"eps_{m}_{qh}_{kk}_{qb}")
                        # two heads concurrently: row tiles (0,0) and (64,0)
                        nc.tensor.matmul(eps[:, 0, :], kT_sb[0:64, m, ks],
                                         qT_sb[0:64, m, qs],
                                         start=True, stop=True)
                        nc.tensor.matmul(eps[:, 1, :], kT_sb[64:128, m, ks],
                                         qT_sb[64:128, m, qs],
                                         start=True, stop=True)
                        e_ps.append(eps)
                    for qb in range(2):
                        e_sb = esb_pool.tile([128, 2, 512], BF16, tag="esb",
                                             name=f"esb_{m}_{qh}_{kk}_{qb}")
                        nc.scalar.activation(e_sb[:], e_ps[qb][:], EXP,
                                             scale=SCALE)
                        e_sbs.append(e_sb)
                    for qb in range(2):
                        # AV pair: col tiles (0,0) and (0,64) concurrent
                        nc.tensor.matmul(
                            avs[qb][0:64, :],
                            v_sb[:, kk, (2 * m) * D:(2 * m) * D + D],
                            e_sbs[qb][:, 0, :], start=first, stop=last)
                        nc.tensor.matmul(
                            avs[qb][64:128, :],
                            v_sb[:, kk, (2 * m + 1) * D:(2 * m + 1) * D + D],
                            e_sbs[qb][:, 1, :], start=first, stop=last)
                    # denominators: 4 concurrent M=1 col tiles at 0/32/64/96
                    for qb in range(2):
                        for h in range(2):
                            r = qb * 64 + h * 32
                            nc.tensor.matmul(d_ps[r:r + 1, :], ones_sb[:],
                                             e_sbs[qb][:, h, :],
                                             start=first, stop=last,
                                             tile_position=(0, r))
                # ---- drain + normalize (drains first: free psum fast) ----
                avsts = []
                for qb in range(2):
                    avst = sm_pool.tile([128, 512], F32, tag="avst", bufs=4,
                                        name=f"avst_{m}_{qh}_{qb}")
                    nc.vector.tensor_copy(avst[:], avs[qb][:])
                    avsts.append(avst)
                dsb = sm_pool.tile([128, 512], F32, tag="dsb", bufs=2,
                                   name=f"dsb_{m}_{qh}")
                nc.vector.tensor_copy(dsb[:], d_ps[:])
                # one 128-partition reciprocal covers all 4 denominator rows
                # (DVE cost scales with free dim only; garbage rows unused)
                dre = sm_pool.tile([128, 512], F32, tag="dre", bufs=2,
                                   name=f"dre_{m}_{qh}")
                nc.vector.reciprocal(dre[:], dsb[:])
                # partition_broadcast only supports src AND dst at base 0:
                # stage each 1/l row at base 0, broadcast a full 128-row
                # tile, and slice the aligned half in the multiply
                rbs = {}
                for qb in range(2):
                    for h in range(2):
                        r = qb * 64 + h * 32
                        rec = sm_pool.tile([1, 512], F32, tag="rec", bufs=8,
                                           name=f"rec_{m}_{qh}_{qb}_{h}")
                        nc.vector.tensor_copy(rec[:], dre[r:r + 1, :])
                        rb = sm_pool.tile([128, 512], F32, tag="rb", bufs=8,
                                          name=f"rb_{m}_{qh}_{qb}_{h}")
                        nc.gpsimd.partition_broadcast(rb[:], rec[:])
                        rbs[qb, h] = rb
                for qb in range(2):
                    qs = slice((2 * qh + qb) * 512, (2 * qh + qb + 1) * 512)
                    for h in range(2):
                        nc.vector.tensor_mul(
                            attT_sb[h * 64:h * 64 + 64, m, qs],
                            avsts[qb][h * 64:h * 64 + 64, :],
                            rbs[qb, h][h * 64:h * 64 + 64, :])

        # ---------- phase 3: remaining output projection ----------
        while fillers:
            pump(1)
        for s in range(NK // 2, NK):
            proj_block(s)


_CACHE = {}


def _build():
    if "nc" not in _CACHE:
        nc = bacc.Bacc("TRN2", target_bir_lowering=False, debug=False,
                       num_devices=NCORES)
        with tile.TileContext(nc) as tc:
            _emit(tc)
        nc.compile()
        _CACHE["nc"] = nc
    return _CACHE["nc"]


def _in_maps(x, W_qkv, W_proj):
    in_maps = []
    for c in range(NCORES):
        b, g = divmod(c, 2)
        r0 = g * HD
        w_rows = np.concatenate([
            W_qkv[0 * EMB + r0: 0 * EMB + r0 + HD],
            W_qkv[1 * EMB + r0: 1 * EMB + r0 + HD],
            W_qkv[2 * EMB + r0: 2 * EMB + r0 + HD],
        ], axis=0)                                   # [1152, 768]
        bf = ml_dtypes.bfloat16
        in_maps.append({
            "xT": np.ascontiguousarray(x[b].T.astype(bf)),
            "wT": np.ascontiguousarray(w_rows.T.astype(bf)),
            "wpT": np.ascontiguousarray(W_proj[:, r0:r0 + HD].T.astype(bf)),
        })
    return in_maps


LAST_RESULTS = None


def kernel(x, W_qkv, W_proj, b_proj):
    global LAST_RESULTS
    x = np.ascontiguousarray(np.asarray(x, dtype=np.float32))
    W_qkv = np.asarray(W_qkv, dtype=np.float32)
    W_proj = np.asarray(W_proj, dtype=np.float32)
    b_proj = np.asarray(b_proj, dtype=np.float32)

    nc = _build()
    in_maps = _in_maps(x, W_qkv, W_proj)
    res = run_bass_kernel_spmd(nc, in_maps, core_ids=list(range(NCORES)))
    LAST_RESULTS = res

    out = np.empty((B, N, EMB), dtype=np.float32)
    for b in range(B):
        out[b] = res.results[2 * b]["outp"] + res.results[2 * b + 1]["outp"]
    out += b_proj
    return out


# revision 16
# speedup vs baseline: 1.4806x; 1.4806x over previous
"""MultiHeadAttention Trainium2 kernel.

Full inputs: x [4, 2048, 768] f32, W_qkv [2304, 768], W_proj [768, 768],
b_proj [768]. Output [4, 2048, 768] f32.

Sharding: 8 cores = 4 batches x 2 head-groups (6 heads each).
Per-core inputs (host-prepared, transposed on host):
  xT  [768, 2048]  = x[b].T
  wT  [768, 1152]  = concat(Wq_g, Wk_g, Wv_g).T   (g = head group rows)
  wpT [384, 768]   = W_proj[:, g-cols].T
Per-core output: outp [2048, 768] = partial projection output for batch b.
Host: out[b] = outp[2b] + outp[2b+1] + b_proj.

On-device (per core), ACT(exp)-bound pipeline:
  The softmax exp volume (6 heads x 2048^2 = 25.2M elems) makes the scalar
  engine the bottleneck (~220us at 1 elem/cycle + per-call overhead), so the
  design keeps ACT 100% busy and hides ALL matmul work underneath it:
  - QK energies: two heads computed CONCURRENTLY on the PE array via row
    tiling (K=64 each: head 2m on array rows 0-63, head 2m+1 on rows 64-127;
    tile_position auto-derived from base partitions).
  - AV: two heads concurrent via column tiling (M=64 each: outputs at psum
    partitions 0-63 / 64-127 of one bank).
  - softmax denominators: four M=1 ones-matmuls (2 heads x 2 q-blocks) run
    concurrently in one PE pass via column tiling at positions 0/32/64/96.
  - PSUM (8 banks): e_ps 2x2 (double-buffered exp staging) + av 2 + denom 1
    + 1 filler bank used to interleave next-pair QKV projections and
    prev-pair output projections into the PE gaps of the ACT-bound stream.
  Loop: for pair m (2 heads): for q-half: for kk (16 key blocks of 128):
    QK pair -> exp (2 ACT calls of [128,2,512]) -> AV pair + denom pass.
  Normalization: drain av/denoms (DVE), reciprocal, gpsimd partition
  broadcast, multiply into attT. Projection out = attT.T @ wpT per s-block.
"""

import ml_dtypes
import numpy as np

import concourse.bass as bass
import concourse.tile as tile
from concourse import bacc, mybir
from concourse.bass_utils import run_bass_kernel_spmd

EMB = 768
N = 2048
B = 4
D = 64
HL = 6            # heads per core
HD = HL * D       # 384 local head-dim columns
NCORES = 8
SCALE = D ** -0.5

F32 = mybir.dt.float32
BF16 = mybir.dt.bfloat16

EC = EMB // 128   # 6 emb chunks
MC = HD // 128    # 3 local head-dim chunks (= head pairs)
NQ = N // 512     # 4 query chunks of 512
NK = N // 128     # 16 key/seq chunks of 128

EXP = mybir.ActivationFunctionType.Exp


def _emit(tc):
    from contextlib import ExitStack

    nc = tc.nc
    xT = nc.dram_tensor("xT", [EMB, N], BF16, kind="ExternalInput").ap()
    wT = nc.dram_tensor("wT", [EMB, 3 * HD], BF16, kind="ExternalInput").ap()
    wpT = nc.dram_tensor("wpT", [HD, EMB], BF16, kind="ExternalInput").ap()
    outp = nc.dram_tensor("outp", [N, EMB], F32, kind="ExternalOutput").ap()

    xTr = xT.rearrange("(c p) s -> p c s", p=128)
    wTr = wT.rearrange("(c p) s -> p c s", p=128)
    wpTr = wpT.rearrange("(m p) e -> p m e", p=128)
    outr = outp.rearrange("(s p) e -> p s e", p=128)

    with ExitStack() as persist:
        ppool = persist.enter_context(tc.tile_pool(name="persist", bufs=1))
        # PE warmup junk matmuls run during the input-DMA wait and open the
        # HAM clock-gate before real work
        warm_sb = ppool.tile([128, 640], BF16)
        nc.vector.memset(warm_sb[:], 1.0)
        ones_sb = ppool.tile([128, 1], BF16)
        nc.vector.memset(ones_sb[:], 1.0)

        wp_sb = ppool.tile([128, MC, EMB], BF16)
        nc.sync.dma_start(wp_sb[:], wpTr)
        x_sb = ppool.tile([128, EC, N], BF16)
        w_sb = ppool.tile([128, EC, 3 * HD], BF16)
        for c in range(EC):
            nc.sync.dma_start(w_sb[:, c, :], wTr[:, c, :])
            nc.sync.dma_start(x_sb[:, c, :], xTr[:, c, :])

        # paired layouts: chunk m holds head 2m on partitions 0-63 and head
        # 2m+1 on partitions 64-127 (both q and k; v is [seq, 6*64])
        qT_sb = ppool.tile([128, MC, N], BF16)
        kT_sb = ppool.tile([128, MC, N], BF16)
        v_sb = ppool.tile([128, NK, HD], BF16)
        attT_sb = ppool.tile([128, MC, N], BF16)

        psum_pool = persist.enter_context(
            tc.tile_pool(name="psum", bufs=1, space="PSUM"))
        esb_pool = persist.enter_context(tc.tile_pool(name="esb", bufs=4))
        sm_pool = persist.enter_context(tc.tile_pool(name="sm", bufs=2))
        osb_pool = persist.enter_context(tc.tile_pool(name="osb", bufs=3))

        warm_ps = psum_pool.tile([128, 512], F32, tag="fill", bufs=1,
                                 name="warm_ps")
        for wi in range(16):
            nc.tensor.matmul(warm_ps[:], warm_sb[:, 0:128], warm_sb[:, 128:640],
                             start=(wi == 0), stop=(wi == 15))

        # ---------- phase-1 building blocks (also used as fillers) ----------
        def qk_block(which, m, n, tag="fill", bufs=1):
            # qT/kT chunk m, q-block n: psum [128, 512] accumulated over EC
            lo = which * HD + m * 128
            ns = slice(n * 512, (n + 1) * 512)
            mm = psum_pool.tile([128, 512], F32, tag=tag, bufs=bufs,
                                name=f"mm_{which}_{m}_{n}")
            for c in range(EC):
                nc.tensor.matmul(mm[:], w_sb[:, c, lo:lo + 128],
                                 x_sb[:, c, ns],
                                 start=(c == 0), stop=(c == EC - 1))
            dst = qT_sb if which == 0 else kT_sb
            nc.vector.tensor_copy(dst[:, m, ns], mm[:])

        def v_block(p, s, tag="fill", bufs=1, width=128):
            # v for pair p (width=256: pairs p and p+1), seq block s
            vv = psum_pool.tile([128, 512], F32, tag=tag, bufs=bufs,
                                name=f"vv_{p}_{s}")[:, 0:width]
            for c in range(EC):
                nc.tensor.matmul(
                    vv[:],
                    x_sb[:, c, s * 128:(s + 1) * 128],
                    w_sb[:, c, 2 * HD + p * 128:2 * HD + p * 128 + width],
                    start=(c == 0), stop=(c == EC - 1))
            nc.vector.tensor_copy(v_sb[:, s, p * 128:p * 128 + width], vv[:])

        def proj_block(s):
            # out[s-block, :] = sum_m attT[:, m, sblk].T @ wp[m]
            o_sb = osb_pool.tile([128, EMB], F32, tag="osb", name=f"osb_{s}")
            ss = slice(s * 128, (s + 1) * 128)
            for half in range(2):
                pr = psum_pool.tile([128, 512], F32, tag="fill", bufs=1,
                                    name=f"pr_{s}_{half}")[:, 0:HD]
                for m in range(MC):
                    nc.tensor.matmul(
                        pr[:], attT_sb[:, m, ss],
                        wp_sb[:, m, half * HD:(half + 1) * HD],
                        start=(m == 0), stop=(m == MC - 1))
                nc.vector.tensor_copy(o_sb[:, half * HD:(half + 1) * HD],
                                      pr[:])
            nc.sync.dma_start(outr[:, s, :], o_sb[:])

        # ---------- startup: minimum work to begin pair-0 attention ----------
        # attention psum banks are still free; use them for 2-deep pipelining
        for n in range(NQ):
            qk_block(1, 0, n, tag="av", bufs=2)  # kT chunk 0 (all kk needed)
        qk_block(0, 0, 0, tag="av", bufs=2)      # qT chunk 0, first q-half
        qk_block(0, 0, 1, tag="av", bufs=2)
        v_block(0, 0, tag="dps", bufs=1)
        v_block(0, 1, tag="av", bufs=2)

        # filler queue: each entry emits one small PE group when pumped
        fillers = [lambda: qk_block(0, 0, 2), lambda: qk_block(0, 0, 3)]
        for s in range(2, NK):
            fillers.append(lambda s=s: v_block(0, s))
        for p in (1, 2):
            for n in range(NQ):
                fillers.append(lambda n=n, p=p: qk_block(1, p, n))
            for n in range(NQ):
                fillers.append(lambda n=n, p=p: qk_block(0, p, n))
            if p == 1:
                for s in range(NK):
                    fillers.append(lambda s=s: v_block(1, s, width=256))

        def pump(k):
            for _ in range(k):
                if fillers:
                    fillers.pop(0)()

        def emit_normalize(m, qh, avs, d_ps):
            # drains first: free the av/denom psum banks quickly
            avsts = []
            for qb in range(2):
                avst = sm_pool.tile([128, 512], F32, tag="avst", bufs=4,
                                    name=f"avst_{m}_{qh}_{qb}")
                nc.vector.tensor_copy(avst[:], avs[qb][:])
                avsts.append(avst)
            dsb = sm_pool.tile([128, 512], F32, tag="dsb", bufs=2,
                               name=f"dsb_{m}_{qh}")
            nc.vector.tensor_copy(dsb[:], d_ps[:])
            # one 128-partition reciprocal covers all 4 denominator rows
            # (DVE cost scales with free dim only; garbage rows unused)
            dre = sm_pool.tile([128, 512], F32, tag="dre", bufs=2,
                               name=f"dre_{m}_{qh}")
            nc.vector.reciprocal(dre[:], dsb[:])
            # partition_broadcast only supports src and dst at base 0: stage
            # each 1/l row at base 0, broadcast a full 128-row tile, slice
            # the aligned half in the multiply
            rbs = {}
            for qb in range(2):
                for h in range(2):
                    r = qb * 64 + h * 32
                    rec = sm_pool.tile([1, 512], F32, tag="rec", bufs=8,
                                       name=f"rec_{m}_{qh}_{qb}_{h}")
                    nc.vector.tensor_copy(rec[:], dre[r:r + 1, :])
                    rb = sm_pool.tile([128, 512], F32, tag="rb", bufs=8,
                                      name=f"rb_{m}_{qh}_{qb}_{h}")
                    nc.gpsimd.partition_broadcast(rb[:], rec[:])
                    rbs[qb, h] = rb
            for qb in range(2):
                qs = slice((2 * qh + qb) * 512, (2 * qh + qb + 1) * 512)
                for h in range(2):
                    nc.vector.tensor_mul(
                        attT_sb[h * 64:h * 64 + 64, m, qs],
                        avsts[qb][h * 64:h * 64 + 64, :],
                        rbs[qb, h][h * 64:h * 64 + 64, :])

        # ---------- attention: ACT-bound software-pipelined stream ----------
        # global e-steps; at step i: ACT(i) | PE: QK(i+2), AV(i), denom
        steps = [(m, qh, kk, qb)
                 for m in range(MC) for qh in range(2)
                 for kk in range(NK) for qb in range(2)]
        US = 2 * NK  # e-steps per (m, qh) unit

        unit_state = {}

        def get_unit(m, qh):
            if (m, qh) not in unit_state:
                avs = [psum_pool.tile([128, 512], F32, tag="av", bufs=2,
                                      name=f"av_{m}_{qh}_{qb}")
                       for qb in range(2)]
                d_ps = psum_pool.tile([128, 512], F32, tag="dps", bufs=1,
                                      name=f"dps_{m}_{qh}")
                unit_state[m, qh] = (avs, d_ps, {})
            return unit_state[m, qh]

        def emit_qk(step):
            m, qh, kk, qb = step
            ks = slice(kk * 128, (kk + 1) * 128)
            qs = slice((2 * qh + qb) * 512, (2 * qh + qb + 1) * 512)
            eps = psum_pool.tile([128, 2, 512], F32, tag="eps", bufs=2,
                                 name=f"eps_{m}_{qh}_{kk}_{qb}")
            # two heads concurrently: row tiles (0,0) and (64,0)
            nc.tensor.matmul(eps[:, 0, :], kT_sb[0:64, m, ks],
                             qT_sb[0:64, m, qs], start=True, stop=True)
            nc.tensor.matmul(eps[:, 1, :], kT_sb[64:128, m, ks],
                             qT_sb[64:128, m, qs], start=True, stop=True)
            return eps

        eps_of = {0: emit_qk(steps[0]), 1: emit_qk(steps[1])}

        for i, step in enumerate(steps):
            if i == 5 * US:
                # attT chunks 0-2 of the first q-half are now complete by the
                # time these pump: proj can fill the last unit's PE gaps
                for s in range(NK // 2):
                    fillers.append(lambda s=s: proj_block(s))
            m, qh, kk, qb = step
            avs, d_ps, e_sbs = get_unit(m, qh)
            first, last = kk == 0, kk == NK - 1
            e_sb = esb_pool.tile([128, 2, 512], BF16, tag="esb",
                                 name=f"esb_{m}_{qh}_{kk}_{qb}")
            nc.scalar.activation(e_sb[:], eps_of.pop(i)[:], EXP, scale=SCALE)
            e_sbs[qb] = e_sb
            if i + 2 < len(steps):
                eps_of[i + 2] = emit_qk(steps[i + 2])
            # AV pair: col tiles (0,0) and (0,64) concurrent
            nc.tensor.matmul(
                avs[qb][0:64, :],
                v_sb[:, kk, (2 * m) * D:(2 * m) * D + D],
                e_sb[:, 0, :], start=first, stop=last)
            nc.tensor.matmul(
                avs[qb][64:128, :],
                v_sb[:, kk, (2 * m + 1) * D:(2 * m + 1) * D + D],
                e_sb[:, 1, :], start=first, stop=last)
            if qb == 1:
                # denominators: 4 concurrent M=1 col tiles at 0/32/64/96
                for qbb in range(2):
                    for h in range(2):
                        r = qbb * 64 + h * 32
                        nc.tensor.matmul(d_ps[r:r + 1, :], ones_sb[:],
                                         e_sbs[qbb][:, h, :],
                                         start=first, stop=last,
                                         tile_position=(0, r))
            if last and qb == 1:
                emit_normalize(m, qh, avs, d_ps)
                del unit_state[m, qh]
            # fillers: skip the first steps of each unit so the previous
            # unit's normalize burst drains from the DVE queue first
            if i < US:
                pump(1)
            elif i % US >= 8:
                pump(1)

        # ---------- phase 3: remaining output projection ----------
        while fillers:
            pump(1)
        for s in range(NK // 2, NK):
            proj_block(s)


_CACHE = {}


def _build():
    if "nc" not in _CACHE:
        nc = bacc.Bacc("TRN2", target_bir_lowering=False, debug=False,
                       num_devices=NCORES)
        with tile.TileContext(nc) as tc:
            _emit(tc)
        nc.compile()
        _CACHE["nc"] = nc
    return _CACHE["nc"]


def _in_maps(x, W_qkv, W_proj):
    in_maps = []
    for c in range(NCORES):
        b, g = divmod(c, 2)
        r0 = g * HD
        w_rows = np.concatenate([
            W_qkv[0 * EMB + r0: 0 * EMB + r0 + HD],
            W_qkv[1 * EMB + r0: 1 * EMB + r0 + HD],
            W_qkv[2 * EMB + r0: 2 * EMB + r0 + HD],
        ], axis=0)                                   # [1152, 768]
        bf = ml_dtypes.bfloat16
        in_maps.append({
            "xT": np.ascontiguousarray(x[b].T.astype(bf)),
            "wT": np.ascontiguousarray(w_rows.T.astype(bf)),
            "wpT": np.ascontiguousarray(W_proj[:, r0:r0 + HD].T.astype(bf)),
        })
    return in_maps


LAST_RESULTS = None


def kernel(x, W_qkv, W_proj, b_proj):
    global LAST_RESULTS
    x = np.ascontiguousarray(np.asarray(x, dtype=np.float32))
    W_qkv = np.asarray(W_qkv, dtype=np.float32)
    W_proj = np.asarray(W_proj, dtype=np.float32)
    b_proj = np.asarray(b_proj, dtype=np.float32)

    nc = _build()
    in_maps = _in_maps(x, W_qkv, W_proj)
    res = run_bass_kernel_spmd(nc, in_maps, core_ids=list(range(NCORES)))
    LAST_RESULTS = res

    out = np.empty((B, N, EMB), dtype=np.float32)
    for b in range(B):
        out[b] = res.results[2 * b]["outp"] + res.results[2 * b + 1]["outp"]
    out += b_proj
    return out



# revision 20
# speedup vs baseline: 1.4878x; 1.0048x over previous
"""MultiHeadAttention Trainium2 kernel.

Full inputs: x [4, 2048, 768] f32, W_qkv [2304, 768], W_proj [768, 768],
b_proj [768]. Output [4, 2048, 768] f32.

Sharding: 8 cores = 4 batches x 2 head-groups (6 heads each).
Per-core inputs (host-prepared, transposed on host):
  xT  [768, 2048]  = x[b].T
  wT  [768, 1152]  = concat(Wq_g, Wk_g, Wv_g).T   (g = head group rows)
  wpT [384, 768]   = W_proj[:, g-cols].T
Per-core output: outp [2048, 768] = partial projection output for batch b.
Host: out[b] = outp[2b] + outp[2b+1] + b_proj.

On-device (per core), ACT(exp)-bound pipeline:
  The softmax exp volume (6 heads x 2048^2 = 25.2M elems) makes the scalar
  engine the bottleneck (~220us at 1 elem/cycle + per-call overhead), so the
  design keeps ACT 100% busy and hides ALL matmul work underneath it:
  - QK energies: two heads computed CONCURRENTLY on the PE array via row
    tiling (K=64 each: head 2m on array rows 0-63, head 2m+1 on rows 64-127;
    tile_position auto-derived from base partitions).
  - AV: two heads concurrent via column tiling (M=64 each: outputs at psum
    partitions 0-63 / 64-127 of one bank).
  - softmax denominators: four M=1 ones-matmuls (2 heads x 2 q-blocks) run
    concurrently in one PE pass via column tiling at positions 0/32/64/96.
  - PSUM (8 banks): e_ps 2x2 (double-buffered exp staging) + av 2 + denom 1
    + 1 filler bank used to interleave next-pair QKV projections and
    prev-pair output projections into the PE gaps of the ACT-bound stream.
  Loop: for pair m (2 heads): for q-half: for kk (16 key blocks of 128):
    QK pair -> exp (2 ACT calls of [128,2,512]) -> AV pair + denom pass.
  Normalization: drain av/denoms (DVE), reciprocal, gpsimd partition
  broadcast, multiply into attT. Projection out = attT.T @ wpT per s-block.
"""

import ml_dtypes
import numpy as np

import concourse.bass as bass
import concourse.tile as tile
from concourse import bacc, mybir
from concourse.bass_utils import run_bass_kernel_spmd

EMB = 768
N = 2048
B = 4
D = 64
HL = 6            # heads per core
HD = HL * D       # 384 local head-dim columns
NCORES = 8
SCALE = D ** -0.5

F32 = mybir.dt.float32
BF16 = mybir.dt.bfloat16

EC = EMB // 128   # 6 emb chunks
MC = HD // 128    # 3 local head-dim chunks (= head pairs)
NQ = N // 512     # 4 query chunks of 512
NK = N // 128     # 16 key/seq chunks of 128

EXP = mybir.ActivationFunctionType.Exp


def _emit(tc):
    from contextlib import ExitStack

    nc = tc.nc
    xT = nc.dram_tensor("xT", [EMB, N], BF16, kind="ExternalInput").ap()
    wT = nc.dram_tensor("wT", [EMB, 3 * HD], BF16, kind="ExternalInput").ap()
    wpT = nc.dram_tensor("wpT", [HD, EMB], BF16, kind="ExternalInput").ap()
    outp = nc.dram_tensor("outp", [N, EMB], F32, kind="ExternalOutput").ap()

    xTr = xT.rearrange("(c p) s -> p c s", p=128)
    wTr = wT.rearrange("(c p) s -> p c s", p=128)
    wpTr = wpT.rearrange("(m p) e -> p m e", p=128)
    outr = outp.rearrange("(s p) e -> p s e", p=128)

    with ExitStack() as persist:
        ppool = persist.enter_context(tc.tile_pool(name="persist", bufs=1))
        # PE warmup junk matmuls run during the input-DMA wait and open the
        # HAM clock-gate before real work
        warm_sb = ppool.tile([128, 640], BF16)
        nc.vector.memset(warm_sb[:], 1.0)
        ones_sb = ppool.tile([128, 1], BF16)
        nc.vector.memset(ones_sb[:], 1.0)
        # preload the exp table set (~2.7us) during the input-DMA wait
        warm_act = ppool.tile([1, 16], BF16)
        nc.scalar.activation(warm_act[:], warm_sb[0:1, 0:16], EXP, scale=1.0)

        wp_sb = ppool.tile([128, MC, EMB], BF16)
        nc.sync.dma_start(wp_sb[:], wpTr)
        x_sb = ppool.tile([128, EC, N], BF16)
        w_sb = ppool.tile([128, EC, 3 * HD], BF16)
        for c in range(EC):
            nc.sync.dma_start(w_sb[:, c, :], wTr[:, c, :])
            nc.sync.dma_start(x_sb[:, c, :], xTr[:, c, :])

        # paired layouts: chunk m holds head 2m on partitions 0-63 and head
        # 2m+1 on partitions 64-127 (both q and k; v is [seq, 6*64])
        qT_sb = ppool.tile([128, MC, N], BF16)
        kT_sb = ppool.tile([128, MC, N], BF16)
        v_sb = ppool.tile([128, NK, HD], BF16)
        attT_sb = ppool.tile([128, MC, N], BF16)

        psum_pool = persist.enter_context(
            tc.tile_pool(name="psum", bufs=1, space="PSUM"))
        esb_pool = persist.enter_context(tc.tile_pool(name="esb", bufs=4))
        sm_pool = persist.enter_context(tc.tile_pool(name="sm", bufs=2))
        osb_pool = persist.enter_context(tc.tile_pool(name="osb", bufs=3))

        warm_ps = psum_pool.tile([128, 512], F32, tag="fill", bufs=1,
                                 name="warm_ps")
        for wi in range(16):
            nc.tensor.matmul(warm_ps[:], warm_sb[:, 0:128], warm_sb[:, 128:640],
                             start=(wi == 0), stop=(wi == 15))

        # ---------- phase-1 building blocks (also used as fillers) ----------
        def qk_block(which, m, n, tag="fill", bufs=1):
            # qT/kT chunk m, q-block n: psum [128, 512] accumulated over EC
            lo = which * HD + m * 128
            ns = slice(n * 512, (n + 1) * 512)
            mm = psum_pool.tile([128, 512], F32, tag=tag, bufs=bufs,
                                name=f"mm_{which}_{m}_{n}")
            for c in range(EC):
                nc.tensor.matmul(mm[:], w_sb[:, c, lo:lo + 128],
                                 x_sb[:, c, ns],
                                 start=(c == 0), stop=(c == EC - 1))
            dst = qT_sb if which == 0 else kT_sb
            nc.vector.tensor_copy(dst[:, m, ns], mm[:])

        def v_block(p, s, tag="fill", bufs=1, width=128):
            # v for pair p (width=256: pairs p and p+1), seq block s
            vv = psum_pool.tile([128, 512], F32, tag=tag, bufs=bufs,
                                name=f"vv_{p}_{s}")[:, 0:width]
            for c in range(EC):
                nc.tensor.matmul(
                    vv[:],
                    x_sb[:, c, s * 128:(s + 1) * 128],
                    w_sb[:, c, 2 * HD + p * 128:2 * HD + p * 128 + width],
                    start=(c == 0), stop=(c == EC - 1))
            nc.vector.tensor_copy(v_sb[:, s, p * 128:p * 128 + width], vv[:])

        def proj_block(s):
            # out[s-block, :] = sum_m attT[:, m, sblk].T @ wp[m]
            o_sb = osb_pool.tile([128, EMB], F32, tag="osb", name=f"osb_{s}")
            ss = slice(s * 128, (s + 1) * 128)
            for half in range(2):
                pr = psum_pool.tile([128, 512], F32, tag="fill", bufs=1,
                                    name=f"pr_{s}_{half}")[:, 0:HD]
                for m in range(MC):
                    nc.tensor.matmul(
                        pr[:], attT_sb[:, m, ss],
                        wp_sb[:, m, half * HD:(half + 1) * HD],
                        start=(m == 0), stop=(m == MC - 1))
                nc.vector.tensor_copy(o_sb[:, half * HD:(half + 1) * HD],
                                      pr[:])
            nc.sync.dma_start(outr[:, s, :], o_sb[:])

        # ---------- startup: minimum work to begin pair-0 attention ----------
        # attention psum banks are still free; use them for 2-deep pipelining
        qk_block(1, 0, 0, tag="av", bufs=2)      # kT chunk 0 block 0 (kk 0-3)
        qk_block(0, 0, 0, tag="av", bufs=2)      # qT chunk 0, first q-half
        qk_block(0, 0, 1, tag="av", bufs=2)
        v_block(0, 0, tag="dps", bufs=1)
        v_block(0, 1, tag="av", bufs=2)

        # filler queue: (due_step, emit_fn) pumped under the ACT stream.
        # Whole blocks are split into halves so one pump stays ~<1us of PE.
        fillq = []

        def add_qk(which, m, n, due):
            state = {}
            lo = which * HD + m * 128
            ns = slice(n * 512, (n + 1) * 512)

            def half_a():
                state["mm"] = psum_pool.tile([128, 512], F32, tag="fill",
                                             bufs=1, name=f"mm_{which}_{m}_{n}")
                for c in range(3):
                    nc.tensor.matmul(state["mm"][:], w_sb[:, c, lo:lo + 128],
                                     x_sb[:, c, ns], start=(c == 0), stop=False)

            def half_b():
                for c in range(3, EC):
                    nc.tensor.matmul(state["mm"][:], w_sb[:, c, lo:lo + 128],
                                     x_sb[:, c, ns], start=False,
                                     stop=(c == EC - 1))
                dst = qT_sb if which == 0 else kT_sb
                nc.vector.tensor_copy(dst[:, m, ns], state["mm"][:])

            fillq.append((due, half_a))
            fillq.append((due, half_b))

        for n in range(1, NQ):                       # kT(0,n): kk block 4n
            add_qk(1, 0, n, 8 * n - 4)
        fillq.append((2, lambda: v_block(0, 2)))
        fillq.append((4, lambda: v_block(0, 3)))
        for s in range(4, NK):
            fillq.append((2 * s - 2, lambda s=s: v_block(0, s)))
        add_qk(0, 0, 2, 28)
        add_qk(0, 0, 3, 28)
        for n in range(NQ):                          # kT(1,n)
            add_qk(1, 1, n, 58 + 8 * n)
        add_qk(0, 1, 0, 60)
        add_qk(0, 1, 1, 60)
        for s in range(NK):
            fillq.append((60 + 2 * s, lambda s=s: v_block(1, s)))
        add_qk(0, 1, 2, 92)
        add_qk(0, 1, 3, 92)
        for n in range(NQ):                          # kT(2,n)
            add_qk(1, 2, n, 122 + 8 * n)
        add_qk(0, 2, 0, 124)
        add_qk(0, 2, 1, 124)
        for s in range(NK):
            fillq.append((124 + 2 * s, lambda s=s: v_block(2, s)))
        add_qk(0, 2, 2, 156)
        add_qk(0, 2, 3, 156)
        fillq.sort(key=lambda t: t[0])

        def add_proj(s):
            state = {}
            ss = slice(s * 128, (s + 1) * 128)

            def half(hf):
                def go():
                    if hf == 0:
                        state["o"] = osb_pool.tile([128, EMB], F32, tag="osb",
                                                   name=f"osb_{s}")
                    pr = psum_pool.tile([128, 512], F32, tag="fill", bufs=1,
                                        name=f"pr_{s}_{hf}")[:, 0:HD]
                    for m in range(MC):
                        nc.tensor.matmul(
                            pr[:], attT_sb[:, m, ss],
                            wp_sb[:, m, hf * HD:(hf + 1) * HD],
                            start=(m == 0), stop=(m == MC - 1))
                    nc.vector.tensor_copy(
                        state["o"][:, hf * HD:(hf + 1) * HD], pr[:])
                    if hf == 1:
                        nc.sync.dma_start(outr[:, s, :], state["o"][:])
                return go

            fillq.append((10 ** 9, half(0)))
            fillq.append((10 ** 9, half(1)))

        def pump_step(i):
            # forced: everything due within the 2-step QK lookahead
            while fillq and fillq[0][0] <= i + 2:
                fillq.pop(0)[1]()
            # optional: one half-block per other step, avoiding the start of
            # a unit where the previous normalize burst clogs the DVE queue
            if fillq and i % 2 == 0 and (i < US or i % US >= 6):
                fillq.pop(0)[1]()

        def emit_normalize(m, qh, avs, d_ps):
            # drains first: free the av/denom psum banks quickly
            avsts = []
            for qb in range(2):
                avst = sm_pool.tile([128, 512], F32, tag="avst", bufs=4,
                                    name=f"avst_{m}_{qh}_{qb}")
                nc.vector.tensor_copy(avst[:], avs[qb][:])
                avsts.append(avst)
            dsb = sm_pool.tile([128, 512], F32, tag="dsb", bufs=2,
                               name=f"dsb_{m}_{qh}")
            nc.vector.tensor_copy(dsb[:], d_ps[:])
            # one 128-partition reciprocal covers all 4 denominator rows
            # (DVE cost scales with free dim only; garbage rows unused)
            dre = sm_pool.tile([128, 512], F32, tag="dre", bufs=2,
                               name=f"dre_{m}_{qh}")
            nc.vector.reciprocal(dre[:], dsb[:])
            # partition_broadcast only supports src and dst at base 0: stage
            # each 1/l row at base 0, broadcast a full 128-row tile, slice
            # the aligned half in the multiply
            rbs = {}
            for qb in range(2):
                for h in range(2):
                    r = qb * 64 + h * 32
                    rec = sm_pool.tile([1, 512], F32, tag="rec", bufs=8,
                                       name=f"rec_{m}_{qh}_{qb}_{h}")
                    nc.vector.tensor_copy(rec[:], dre[r:r + 1, :])
                    rb = sm_pool.tile([128, 512], F32, tag="rb", bufs=8,
                                      name=f"rb_{m}_{qh}_{qb}_{h}")
                    nc.gpsimd.partition_broadcast(rb[:], rec[:])
                    rbs[qb, h] = rb
            for qb in range(2):
                qs = slice((2 * qh + qb) * 512, (2 * qh + qb + 1) * 512)
                for h in range(2):
                    nc.vector.tensor_mul(
                        attT_sb[h * 64:h * 64 + 64, m, qs],
                        avsts[qb][h * 64:h * 64 + 64, :],
                        rbs[qb, h][h * 64:h * 64 + 64, :])

        # ---------- attention: ACT-bound software-pipelined stream ----------
        # global e-steps; at step i: ACT(i) | PE: QK(i+2), AV(i), denom
        steps = [(m, qh, kk, qb)
                 for m in range(MC) for qh in range(2)
                 for kk in range(NK) for qb in range(2)]
        US = 2 * NK  # e-steps per (m, qh) unit

        unit_state = {}

        def get_unit(m, qh):
            if (m, qh) not in unit_state:
                avs = [psum_pool.tile([128, 512], F32, tag="av", bufs=2,
                                      name=f"av_{m}_{qh}_{qb}")
                       for qb in range(2)]
                d_ps = psum_pool.tile([128, 512], F32, tag="dps", bufs=1,
                                      name=f"dps_{m}_{qh}")
                unit_state[m, qh] = (avs, d_ps, {})
            return unit_state[m, qh]

        def emit_qk(step):
            m, qh, kk, qb = step
            ks = slice(kk * 128, (kk + 1) * 128)
            qs = slice((2 * qh + qb) * 512, (2 * qh + qb + 1) * 512)
            eps = psum_pool.tile([128, 2, 512], F32, tag="eps", bufs=2,
                                 name=f"eps_{m}_{qh}_{kk}_{qb}")
            # two heads concurrently: row tiles (0,0) and (64,0)
            nc.tensor.matmul(eps[:, 0, :], kT_sb[0:64, m, ks],
                             qT_sb[0:64, m, qs], start=True, stop=True)
            nc.tensor.matmul(eps[:, 1, :], kT_sb[64:128, m, ks],
                             qT_sb[64:128, m, qs], start=True, stop=True)
            return eps

        eps_of = {0: emit_qk(steps[0]), 1: emit_qk(steps[1])}

        for i, step in enumerate(steps):
            if i == 5 * US:
                # attT chunks 0-2 of the first q-half are now complete by the
                # time these pump: proj can fill the last unit's PE gaps
                for s in range(NK // 2):
                    add_proj(s)
            m, qh, kk, qb = step
            avs, d_ps, e_sbs = get_unit(m, qh)
            first, last = kk == 0, kk == NK - 1
            e_sb = esb_pool.tile([128, 2, 512], BF16, tag="esb",
                                 name=f"esb_{m}_{qh}_{kk}_{qb}")
            nc.scalar.activation(e_sb[:], eps_of.pop(i)[:], EXP, scale=SCALE)
            e_sbs[qb] = e_sb
            if i + 2 < len(steps):
                eps_of[i + 2] = emit_qk(steps[i + 2])
            # AV pair: col tiles (0,0) and (0,64) concurrent
            nc.tensor.matmul(
                avs[qb][0:64, :],
                v_sb[:, kk, (2 * m) * D:(2 * m) * D + D],
                e_sb[:, 0, :], start=first, stop=last)
            nc.tensor.matmul(
                avs[qb][64:128, :],
                v_sb[:, kk, (2 * m + 1) * D:(2 * m + 1) * D + D],
                e_sb[:, 1, :], start=first, stop=last)
            if qb == 1:
                # denominators: 4 concurrent M=1 col tiles at 0/32/64/96
                for qbb in range(2):
                    for h in range(2):
                        r = qbb * 64 + h * 32
                        nc.tensor.matmul(d_ps[r:r + 1, :], ones_sb[:],
                                         e_sbs[qbb][:, h, :],
                                         start=first, stop=last,
                                         tile_position=(0, r))
            if last and qb == 1:
                emit_normalize(m, qh, avs, d_ps)
                del unit_state[m, qh]
            pump_step(i)

        # ---------- phase 3: remaining output projection ----------
        while fillq:
            fillq.pop(0)[1]()
        # tail projections pipeline through the now-free attention banks
        tags = [("av", 2), ("dps", 1), ("fill", 1)]
        for j, s in enumerate(range(NK // 2, NK)):
            tag, bufs = tags[j % 3]
            o_sb = osb_pool.tile([128, EMB], F32, tag="osb", name=f"osb_{s}")
            ss = slice(s * 128, (s + 1) * 128)
            for half in range(2):
                pr = psum_pool.tile([128, 512], F32, tag=tag, bufs=bufs,
                                    name=f"pr_{s}_{half}")[:, 0:HD]
                for m in range(MC):
                    nc.tensor.matmul(
                        pr[:], attT_sb[:, m, ss],
                        wp_sb[:, m, half * HD:(half + 1) * HD],
                        start=(m == 0), stop=(m == MC - 1))
                nc.vector.tensor_copy(o_sb[:, half * HD:(half + 1) * HD],
                                      pr[:])
            nc.sync.dma_start(outr[:, s, :], o_sb[:])


_CACHE = {}


def _build():
    if "nc" not in _CACHE:
        nc = bacc.Bacc("TRN2", target_bir_lowering=False, debug=False,
                       num_devices=NCORES)
        with tile.TileContext(nc) as tc:
            _emit(tc)
        nc.compile()
        _CACHE["nc"] = nc
    return _CACHE["nc"]


def _in_maps(x, W_qkv, W_proj):
    in_maps = []
    for c in range(NCORES):
        b, g = divmod(c, 2)
        r0 = g * HD
        w_rows = np.concatenate([
            W_qkv[0 * EMB + r0: 0 * EMB + r0 + HD],
            W_qkv[1 * EMB + r0: 1 * EMB + r0 + HD],
            W_qkv[2 * EMB + r0: 2 * EMB + r0 + HD],
        ], axis=0)                                   # [1152, 768]
        bf = ml_dtypes.bfloat16
        in_maps.append({
            "xT": np.ascontiguousarray(x[b].T.astype(bf)),
            "wT": np.ascontiguousarray(w_rows.T.astype(bf)),
            "wpT": np.ascontiguousarray(W_proj[:, r0:r0 + HD].T.astype(bf)),
        })
    return in_maps


LAST_RESULTS = None


def kernel(x, W_qkv, W_proj, b_proj):
    global LAST_RESULTS
    x = np.ascontiguousarray(np.asarray(x, dtype=np.float32))
    W_qkv = np.asarray(W_qkv, dtype=np.float32)
    W_proj = np.asarray(W_proj, dtype=np.float32)
    b_proj = np.asarray(b_proj, dtype=np.float32)

    nc = _build()
    in_maps = _in_maps(x, W_qkv, W_proj)
    res = run_bass_kernel_spmd(nc, in_maps, core_ids=list(range(NCORES)))
    LAST_RESULTS = res

    out = np.empty((B, N, EMB), dtype=np.float32)
    for b in range(B):
        out[b] = res.results[2 * b]["outp"] + res.results[2 * b + 1]["outp"]
    out += b_proj
    return out

